# revision 41
# baseline (speedup 1.0000x reference)
"""Trainium2 Bass kernel: LayerNorm + multi-head self-attention + residual.

Computes, per batch b:
    xn = LayerNorm(x[b]) * g + b
    q/k/v = xn @ W{q,k,v}.T + b{q,k,v}      (16 heads, dh=64)
    attn  = softmax(q k^T + maskbias, over keys)
    out   = x + (attn @ (v*mask)) reshaped

Sharding over 8 cores: batch (2-way) x head-group (4-way, 4 heads each).
Each core gets full x[b] (for LayerNorm) plus its 256-column slice of the
Q/K/V weights, and produces a [2048, 256] slice of the output.

Host-side folding: LN's g is folded into the weight columns and LN's b into
the projection biases, so the device only computes xc = (x - mu) * rstd.

This version software-pipelines the whole kernel around the ACT engine's
softmax-exp stream (the densest single-engine load, ~133us):
  - x is streamed in 128-row chunks; LN stats + affine on DVE; rstd is a
    batched (4 chunks/window) Quake-seed + 2-Newton-step rsqrt on DVE so
    ACT never needs Sqrt/Ln — every ACT func used (Exp, Identity, Copy)
    lives in the first activation-table set, so there are no table swaps
    inside the interleaved exp stream.
  - Q/K/V projection PSUM drains run on ACT (Identity with per-partition
    bias AP / Copy with per-partition scale AP), using phase-1 ACT slack.
  - xn^T is produced by DMA-xbar transposes (dma_start_transpose),
    removing the PE transpose + PSUM->SBUF copy traffic entirely.
  - phase 1 (per 4-chunk window w): LN chunks, K/V projections for the
    window, plus the j4=0 attention window of BOTH head pairs running at
    LN pace so ACT starts exp'ing at ~10us.
  - phase 2: remaining 6 attention windows (n-windows of 512, head pairs
    A|B packed in one [128,1024] PSUM tile so each EXP instruction covers
    1024 elements), pure ACT-bound; Q projections for window j4+1 are
    slipped into round j4 as PE filler.
  - PSUM: tag "s" 2x[128,1024] (S tiles + all projection / transpose
    scratch) + tag "y" 4x[128,512] (two concurrent attention windows'
    Y accumulators) = exactly 8 banks.
  - V' carries a ones column per head so the AV matmul also produces the
    softmax denominators (row 64 of Y^T); epilogue PE-transposes Y^T,
    normalizes, adds the residual and DMAs out per window.

Precision: matmul operands fp16, softmax weights bf16, fp32 accumulation.
"""

import sys

for _p in ("/opt/trn_rl_repo",):
    if _p not in sys.path:
        sys.path.insert(0, _p)

import numpy as np

import concourse.bacc as bacc
import concourse.bass as bass
import concourse.mybir as mybir
import concourse.tile as tile
from concourse.masks import make_identity

F32 = mybir.dt.float32
F16 = mybir.dt.float16
BF16 = mybir.dt.bfloat16

T = 2048          # sequence length
D = 1024          # model dim
HC = 4            # heads per core
DH = 64           # head dim
CC = HC * DH      # columns per core (256)
NC = T // 128     # 16 n/m chunks of 128
DC = D // 128     # 8 d chunks
W = 512           # attention n-window width
NWIN = T // W     # 4 n-windows per head pair

_CACHE = {}


def build_bass():
    # Bacc (not plain Bass): its finalize() runs generate_event_semaphores,
    # which splits multi-waits into EventSemaphore instructions — walrus
    # rejects >1 sync wait on most engine instruction structs.
    nc = bacc.Bacc()

    x_d = nc.declare_dram_parameter("x", [T, D], F32, isOutput=False)
    xres_d = nc.declare_dram_parameter("xres", [T, CC], F32, isOutput=False)
    wqt_d = nc.declare_dram_parameter("wqt", [D, CC], F16, isOutput=False)
    wkt_d = nc.declare_dram_parameter("wkt", [D, CC], F16, isOutput=False)
    wvt_d = nc.declare_dram_parameter("wvt", [D, CC], F16, isOutput=False)
    bq_d = nc.declare_dram_parameter("bq2", [128, 2], F32, isOutput=False)
    bk_d = nc.declare_dram_parameter("bk2", [128, 2], F32, isOutput=False)
    bvr_d = nc.declare_dram_parameter("bvr", [1, CC], F16, isOutput=False)
    mb_d = nc.declare_dram_parameter("mbias", [128, NC], F32, isOutput=False)
    mm_d = nc.declare_dram_parameter("mmul", [128, NC], F32, isOutput=False)
    out_d = nc.declare_dram_parameter("out", [T, CC], F32, isOutput=True)

    with tile.TileContext(nc) as tc:
        _body(tc, x_d, xres_d, wqt_d, wkt_d, wvt_d,
              bq_d, bk_d, bvr_d, mb_d, mm_d, out_d)
    nc.finalize()
    return nc


def _body(tc, x_d, xres_d, wqt_d, wkt_d, wvt_d,
          bq_d, bk_d, bvr_d, mb_d, mm_d, out_d):
    nc = tc.nc
    import contextlib
    ctx = contextlib.ExitStack()
    with ctx:
        consts = ctx.enter_context(tc.tile_pool(name="consts", bufs=1))
        persist = ctx.enter_context(tc.tile_pool(name="persist", bufs=1))
        xpool = ctx.enter_context(tc.tile_pool(name="xpool", bufs=16))
        xcpool = ctx.enter_context(tc.tile_pool(name="xcpool", bufs=6))
        stats = ctx.enter_context(tc.tile_pool(name="stats", bufs=4))
        ppool = ctx.enter_context(tc.tile_pool(name="ppool", bufs=8))
        ytpool = ctx.enter_context(tc.tile_pool(name="ytpool", bufs=4))
        outpool = ctx.enter_context(tc.tile_pool(name="outpool", bufs=3))
        recpool = ctx.enter_context(tc.tile_pool(name="recpool", bufs=8))
        spsum = ctx.enter_context(tc.tile_pool(name="spsum", bufs=3, space="PSUM"))
        ypsum = ctx.enter_context(tc.tile_pool(name="ypsum", bufs=2, space="PSUM"))

        # ---- persistent activations -----------------------------------
        xnT = persist.tile([128, DC, T], F16)       # xn^T (g,b folded on host)
        qT = persist.tile([128, 2, T], F16)         # Q^T per head-pair
        kT = persist.tile([128, 2, T], F16)
        vP = persist.tile([128, NC, HC * (DH + 1)], BF16)  # V' with ones cols
        xres_all = persist.tile([128, NC, CC], F32)
        vP4 = vP[:].rearrange("p i (h c) -> p i h c", c=DH + 1)

        # ---- computed constants FIRST: they sit on engine queues ahead of
        # the DMA issues, so the first PE transposes aren't blocked behind
        # 20+ queued DMA instructions -------------------------------------
        ident32 = consts.tile([128, 128], F32)
        make_identity(nc, ident32)
        ident16 = consts.tile([128, 128], F16)
        make_identity(nc, ident16)
        ones1 = consts.tile([1, 128], F16)
        nc.vector.memset(ones1, 1.0)
        # Quake rsqrt seed constant 0x5f3759df, stored as the f32 with those
        # bits so integer tensor_tensor ops can read it via bitcast.
        I32 = mybir.dt.int32
        qk_c = consts.tile([128, 4], F32)
        nc.vector.memset(qk_c,
                         float(np.array([0x5F3759DF], np.uint32)
                               .view(np.float32)[0]))

        # ---- input DMAs on the GPSIMD SWDGE queue so the SP queue stays
        # free for the latency-critical xn^T xbar transposes ---------------
        xt = [None] * NC

        def fetch_x(ic):
            xt[ic] = xpool.tile([128, D], F32, tag="x", name=f"xt{ic}")
            nc.gpsimd.dma_start(xt[ic], x_d[128 * ic:128 * (ic + 1), :])

        fetch_x(0)
        fetch_x(1)
        fetch_x(2)
        fetch_x(3)
        bq_t = consts.tile([128, 2], F32)
        bk_t = consts.tile([128, 2], F32)
        nc.gpsimd.dma_start(bq_t, bq_d[:])
        nc.gpsimd.dma_start(bk_t, bk_d[:])
        bvr_t = consts.tile([1, CC], F16)
        nc.gpsimd.dma_start(bvr_t, bvr_d[:])
        mb_t = consts.tile([128, NC], F32)
        mm_t = consts.tile([128, NC], F32)
        nc.gpsimd.dma_start(mb_t, mb_d[:])
        nc.gpsimd.dma_start(mm_t, mm_d[:])
        wk_sb = consts.tile([128, DC, CC], F16)
        nc.gpsimd.dma_start(wk_sb,
                            wkt_d[:].rearrange("(o p) c -> p o c", p=128))
        fetch_x(4)
        fetch_x(5)
        wq_sb = consts.tile([128, DC, CC], F16)
        wv_sb = consts.tile([128, DC, CC], F16)
        nc.gpsimd.dma_start(wq_sb,
                            wqt_d[:].rearrange("(o p) c -> p o c", p=128))
        nc.gpsimd.dma_start(wv_sb,
                            wvt_d[:].rearrange("(o p) c -> p o c", p=128))
        for _ic in range(6, NC):
            fetch_x(_ic)

        # absorb const-DMA completion waits on the engines that later read
        # these tiles via scalar-pointer operands (those instruction structs
        # can encode only one sync wait)
        touch_v = consts.tile([128, 1], F32)
        nc.vector.tensor_copy(touch_v, bq_t[:, 0:1])
        touch_a = consts.tile([128, 1], F32)
        nc.scalar.copy(touch_a, mb_t[:, 0:1])
        nc.scalar.copy(touch_a, mm_t[:, 0:1])
        nc.scalar.copy(touch_a, bq_t[:, 0:1])
        nc.scalar.copy(touch_a, bk_t[:, 0:1])

        # ones columns of V' (softmax denominator trick)
        nc.vector.memset(vP4[:, :, :, DH], 1.0)

        # ---- pipeline steps -------------------------------------------
        def ln_window(w):
            """LN for chunks 4w..4w+3: DVE stats, batched Newton rsqrt,
            affine split DVE/ACT, transpose into xn^T (PE for the first 8
            chunks — low latency while the DMA queue is cold — DMA-xbar for
            the rest, which overlaps compute once streaming)."""
            mvw = stats.tile([128, 2, 4], F32, tag="mv", name="mvw")
            for j, ic in enumerate(range(4 * w, 4 * w + 4)):
                st = stats.tile([128, 2, 6], F32, tag="st", name="st")
                nc.vector.bn_stats(st[:, 0, :], xt[ic][:, 0:512])
                nc.vector.bn_stats(st[:, 1, :], xt[ic][:, 512:1024])
                nc.vector.bn_aggr(mvw[:, :, j], st)
            # rstd = rsqrt(var + eps), batched over the 4 chunks
            varw = stats.tile([128, 4], F32, tag="var", name="varw")
            rsw = stats.tile([128, 4], F32, tag="rs", name="rsw")
            tn = stats.tile([128, 4], F32, tag="tn", name="tn")
            nc.vector.tensor_scalar_add(varw, mvw[:, 1, :], 1e-5)
            nc.vector.tensor_scalar(
                out=tn.bitcast(I32), in0=varw.bitcast(I32), scalar1=1,
                scalar2=None, op0=mybir.AluOpType.logical_shift_right)
            nc.vector.tensor_tensor(
                out=rsw.bitcast(I32), in0=qk_c.bitcast(I32),
                in1=tn.bitcast(I32), op=mybir.AluOpType.subtract)
            for _ in range(2):
                nc.vector.tensor_tensor(out=tn, in0=rsw, in1=rsw,
                                        op=mybir.AluOpType.mult)
                nc.vector.tensor_tensor(out=tn, in0=tn, in1=varw,
                                        op=mybir.AluOpType.mult)
                nc.vector.tensor_scalar(out=tn, in0=tn, scalar1=-0.5,
                                        scalar2=1.5,
                                        op0=mybir.AluOpType.mult,
                                        op1=mybir.AluOpType.add)
                nc.vector.tensor_tensor(out=rsw, in0=rsw, in1=tn,
                                        op=mybir.AluOpType.mult)
            # negated rstd, for the ACT-affine bias (-mu * rstd)
            nrw = stats.tile([128, 4], F32, tag="nr", name="nrw")
            nc.vector.tensor_scalar_mul(nrw, rsw, -1.0)
            for j, ic in enumerate(range(4 * w, 4 * w + 4)):
                xc = xcpool.tile([128, D], F16, tag="xc", name="xc")
                if j % 2 == 0:
                    # DVE affine: (x - mu) * rstd
                    nc.vector.tensor_scalar(
                        out=xc, in0=xt[ic], scalar1=mvw[:, 0, j:j + 1],
                        scalar2=rsw[:, j:j + 1],
                        op0=mybir.AluOpType.subtract,
                        op1=mybir.AluOpType.mult)
                else:
                    # ACT affine: x * rstd + (-mu * rstd), using phase-1
                    # ACT slack (Identity is in the Exp table set)
                    bln = stats.tile([128, 1], F32, tag="bln", name="bln")
                    nc.vector.tensor_tensor(
                        out=bln, in0=mvw[:, 0, j:j + 1],
                        in1=nrw[:, j:j + 1], op=mybir.AluOpType.mult)
                    nc.scalar.activation(
                        xc, xt[ic], mybir.ActivationFunctionType.Identity,
                        bias=bln, scale=rsw[:, j:j + 1])
                if ic < 8:
                    # PE transpose into a PSUM bank (f16-bitcast); the
                    # scatter into xn^T alternates DVE / ACT to balance load
                    tps = spsum.tile([128, 1024], F32, tag="s",
                                     name="tps")[:, 0:512].bitcast(F16)
                    for k in range(DC):
                        nc.tensor.transpose(tps[:, 128 * k:128 * (k + 1)],
                                            xc[:, 128 * k:128 * (k + 1)],
                                            ident16)
                    nc.vector.tensor_copy(
                        xnT[:, :, 128 * ic:128 * (ic + 1)],
                        tps.rearrange("p (o c) -> p o c", c=128))
                else:
                    # DMA-xbar transpose (SP queue only: ACT-issued
                    # transposes corrupt data on HW)
                    nc.sync.dma_start_transpose(
                        xnT[:, :, 128 * ic:128 * (ic + 1)], xc)

        def qk_proj1(w_sb, dstT, b_t, w, pg, drain):
            ps = spsum.tile([128, 1024], F32, tag="s",
                            name=f"pj{w}_{pg}")[:, 0:W]
            for dc in range(DC):
                nc.tensor.matmul(ps,
                                 lhsT=w_sb[:, dc, 128 * pg:128 * (pg + 1)],
                                 rhs=xnT[:, dc, W * w:W * (w + 1)],
                                 start=(dc == 0), stop=(dc == DC - 1))
            if drain == "act":
                # psum->SBUF drain + bias on ACT (Identity is in the same
                # activation-table set as Exp: no table swap)
                nc.scalar.activation(dstT[:, pg, W * w:W * (w + 1)], ps,
                                     mybir.ActivationFunctionType.Identity,
                                     bias=b_t[:, pg:pg + 1], scale=1.0)
            else:
                nc.vector.tensor_scalar_add(
                    out=dstT[:, pg, W * w:W * (w + 1)], in0=ps,
                    scalar1=b_t[:, pg:pg + 1])

        def qk_projw(w_sb, dstT, b_t, w, drain="act"):
            for pg in range(2):
                qk_proj1(w_sb, dstT, b_t, w, pg, drain)

        def v_proj(ic):
            psv = spsum.tile([128, 1024], F32, tag="s", name="psv")[:, 0:CC]
            for dc in range(DC):
                nc.tensor.matmul(psv,
                                 lhsT=xnT[:, dc, 128 * ic:128 * (ic + 1)],
                                 rhs=wv_sb[:, dc, :],
                                 start=(dc == 0), stop=False)
            # rank-1 bias add: ones[1,128].T @ bv[1,CC]
            nc.tensor.matmul(psv, lhsT=ones1, rhs=bvr_t,
                             start=False, stop=True)
            # psum->SBUF drain * mask on ACT (Copy with per-partition scale)
            nc.scalar.activation(vP4[:, ic, :, 0:DH],
                                 psv.rearrange("p (h c) -> p h c", c=DH),
                                 mybir.ActivationFunctionType.Copy,
                                 bias=0.0, scale=mm_t[:, ic:ic + 1])

        # One AV step of lag between S/exp and AV: when round r's last exp
        # runs, round r+1's first S matmuls are already queued on the PE, so
        # the exp stream never waits across a round boundary.
        pending_av = [None]

        def att_step(pg, j4, m, yA, yB):
            sc = spsum.tile([128, 1024], F32, tag="s", name=f"sc{pg}")
            msl = slice(128 * m, 128 * (m + 1))
            nsl = slice(W * j4, W * (j4 + 1))
            nc.tensor.matmul(sc[:, 0:W], lhsT=kT[0:DH, pg, msl],
                             rhs=qT[0:DH, pg, nsl], start=True, stop=True)
            nc.tensor.matmul(sc[:, W:2 * W], lhsT=kT[DH:128, pg, msl],
                             rhs=qT[DH:128, pg, nsl], start=True, stop=True)
            p = ppool.tile([128, 1024], BF16, tag="p", name="p")
            nc.scalar.activation(p, sc,
                                 mybir.ActivationFunctionType.Exp,
                                 bias=mb_t[:, m:m + 1], scale=1.0)
            if pending_av[0] is not None:
                pending_av[0]()

            def av():
                hA, hB = 2 * pg, 2 * pg + 1
                vA = vP[:, m, (DH + 1) * hA:(DH + 1) * (hA + 1)]
                vB = vP[:, m, (DH + 1) * hB:(DH + 1) * (hB + 1)]
                nc.tensor.matmul(yA, lhsT=vA, rhs=p[:, 0:W],
                                 start=(m == 0), stop=(m == NC - 1))
                nc.tensor.matmul(yB, lhsT=vB, rhs=p[:, W:2 * W],
                                 start=(m == 0), stop=(m == NC - 1))
            pending_av[0] = av

        def alloc_y(pg):
            yA = ypsum.tile([128, W], F32, tag="y", name=f"yA{pg}")[0:DH + 1]
            yB = ypsum.tile([128, W], F32, tag="y", name=f"yB{pg}")[0:DH + 1]
            return yA, yB

        def epilogue_steps(pg, j4, yA, yB):
            """Yield the epilogue as small closures so the caller can spread
            them between the next round's att_steps (keeps the PE/DVE bursts
            from stalling the ACT exp stream at round boundaries)."""
            state = {}

            def s_copyA():
                state["ytA"] = ytpool.tile([DH + 1, W], F32, tag="yt",
                                           name="ytA")
                nc.vector.tensor_copy(state["ytA"], yA)

            def s_copyB():
                state["ytB"] = ytpool.tile([DH + 1, W], F32, tag="yt",
                                           name="ytB")
                nc.vector.tensor_copy(state["ytB"], yB)
                state["out_t"] = outpool.tile([128, 4, 128], F32, tag="out",
                                              name="out_t")

            def s_kblock(k):
                def run():
                    out_t = state["out_t"]
                    for hh, yt in ((0, state["ytA"]), (1, state["ytB"])):
                        otp = spsum.tile([128, 1024], F32, tag="s",
                                         name="otp")[:, 0:DH + 1]
                        nc.tensor.transpose(otp, yt[:, 128 * k:128 * (k + 1)],
                                            ident32[0:DH + 1, 0:DH + 1])
                        rec = recpool.tile([128, 1], F32, tag="rec",
                                           name="rec")
                        nc.vector.reciprocal(rec, otp[:, DH:DH + 1])
                        nc.vector.tensor_scalar_mul(
                            out=out_t[:, k, DH * hh:DH * (hh + 1)],
                            in0=otp[:, 0:DH], scalar1=rec)
                    nc.vector.tensor_add(
                        out_t[:, k, :], out_t[:, k, :],
                        xres_all[:, 4 * j4 + k, 128 * pg:128 * (pg + 1)])
                return run

            def s_store():
                nc.sync.dma_start(
                    out_d[W * j4:W * (j4 + 1),
                          128 * pg:128 * (pg + 1)].rearrange(
                              "(o p) c -> p o c", p=128),
                    state["out_t"])

            # index = the m-step of the NEXT round at which each piece runs
            return {0: s_copyA, 1: s_copyB, 3: s_kblock(0), 5: s_kblock(1),
                    7: s_kblock(2), 9: s_kblock(3), 10: s_store}

        # ---------------- phase 1: LN + K/V proj + (pg0, j4=0) window ---
        # The round-0 att_steps trail the LN/proj pipeline by one 4-chunk
        # window so their EXPs never sit ahead of the next window's ACT
        # affines in the in-order ACT queue (that ordering creates a
        # cross-window feedback stall).
        y0 = alloc_y(0)
        for w in range(4):
            ln_window(w)
            qk_projw(wk_sb, kT, bk_t, w)
            if w == 0:
                qk_projw(wq_sb, qT, bq_t, 0)
            if w == 1:
                nc.gpsimd.dma_start(
                    xres_all,
                    xres_d[:].rearrange("(o p) c -> p o c", p=128))
            for ic in range(4 * w, 4 * w + 4):
                v_proj(ic)
            # att_steps follow their own window's projections immediately for
            # the PE-transposed windows (fast path); the DMA-transposed
            # windows (2,3) keep a one-window shift so their exps don't sit
            # ahead of the next window's ACT work while waiting on the
            # slower xbar transposes.
            if w <= 1:
                for m in range(4 * w, 4 * w + 4):
                    att_step(0, 0, m, *y0)
            elif w == 3:
                for m in range(8, 12):
                    att_step(0, 0, m, *y0)
        for m in range(12, 16):
            att_step(0, 0, m, *y0)
        pending_ep = epilogue_steps(0, 0, *y0)

        # ---------------- phase 2: remaining windows, ACT-bound ---------
        # Per round: previous round's epilogue is spread over m=0..6 and the
        # NEXT round's Q projection over m=12..15 (2 dc-matmuls per step, so
        # the PE never blocks the exp stream for more than ~0.5us).
        rounds = [(1, 0), (0, 1), (1, 1), (0, 2), (1, 2), (0, 3), (1, 3)]
        for r, (pg, j4) in enumerate(rounds):
            yw = alloc_y(pg)
            qfill = None
            if r + 1 < len(rounds):
                npg, nj4 = rounds[r + 1]
                if (npg, nj4) != (1, 0):
                    qfill = (npg, nj4)
            qps = None
            for m in range(NC):
                att_step(pg, j4, m, *yw)
                if m in pending_ep:
                    pending_ep[m]()
                if qfill is not None and m >= 12:
                    npg, nj4 = qfill
                    if qps is None:
                        qps = spsum.tile([128, 1024], F32, tag="s",
                                         name=f"qf{r}")[:, 0:W]
                    for dc in (2 * (m - 12), 2 * (m - 12) + 1):
                        nc.tensor.matmul(
                            qps,
                            lhsT=wq_sb[:, dc, 128 * npg:128 * (npg + 1)],
                            rhs=xnT[:, dc, W * nj4:W * (nj4 + 1)],
                            start=(dc == 0), stop=(dc == DC - 1))
            if qfill is not None:
                npg, nj4 = qfill
                nc.vector.tensor_scalar_add(
                    out=qT[:, npg, W * nj4:W * (nj4 + 1)], in0=qps,
                    scalar1=bq_t[:, npg:npg + 1])
            pending_ep = epilogue_steps(pg, j4, *yw)
        pending_av[0]()
        pending_av[0] = None
        for m in sorted(pending_ep):
            pending_ep[m]()


def _host_in_map(core, x, src_mask, ln_g, ln_b, Wq, bq, Wk, bk, Wv, bv):
    b, hg = divmod(core, 4)
    cs = CC * hg
    xb = np.ascontiguousarray(x[b], dtype=np.float32)
    mask = np.asarray(src_mask[b, :, 0], dtype=np.float32)
    ln_g = np.asarray(ln_g, np.float32)
    ln_b = np.asarray(ln_b, np.float32)

    def wfold(W):
        # fold LN scale g into weight columns: (W * g).T, fp16
        Ws = np.asarray(W, np.float32)[cs:cs + CC, :]
        return np.ascontiguousarray((Ws * ln_g[None, :]).T).astype(np.float16)

    def bfold(W, bb):
        # fold LN shift b into the projection bias: W @ b + bias
        Ws = np.asarray(W, np.float32)[cs:cs + CC, :]
        return Ws @ ln_b + np.asarray(bb, np.float32)[cs:cs + CC]

    return {
        "x": xb,
        "xres": np.ascontiguousarray(xb[:, cs:cs + CC]),
        "wqt": wfold(Wq),
        "wkt": wfold(Wk),
        "wvt": wfold(Wv),
        "bq2": np.ascontiguousarray(bfold(Wq, bq).reshape(2, 128).T),
        "bk2": np.ascontiguousarray(bfold(Wk, bk).reshape(2, 128).T),
        "bvr": bfold(Wv, bv).reshape(1, CC).astype(np.float16),
        "mbias": np.ascontiguousarray(
            ((1.0 - mask) * -1000000.0).reshape(NC, 128).T),
        "mmul": np.ascontiguousarray(mask.reshape(NC, 128).T),
    }


def kernel(x, src_mask, ln_g, ln_b, Wq, bq, Wk, bk, Wv, bv, _trace=False,
           _tmpdir=None):
    x = np.asarray(x, dtype=np.float32)
    B = x.shape[0]
    if "nc" not in _CACHE:
        _CACHE["nc"] = build_bass()
    nc = _CACHE["nc"]

    from concourse.bass_utils import run_bass_kernel_spmd
    in_maps = [
        _host_in_map(c, x, np.asarray(src_mask), np.asarray(ln_g),
                     np.asarray(ln_b), np.asarray(Wq), np.asarray(bq),
                     np.asarray(Wk), np.asarray(bk), np.asarray(Wv),
                     np.asarray(bv))
        for c in range(8)
    ]
    res = run_bass_kernel_spmd(nc, in_maps, core_ids=list(range(8)),
                               trace=_trace, tmpdir=_tmpdir)
    out = np.empty((B, T, D), dtype=np.float32)
    for c in range(8):
        b, hg = divmod(c, 4)
        out[b, :, CC * hg:CC * (hg + 1)] = res.results[c]["out"]
    if _trace:
        _CACHE["last_result"] = res
    return out
